# revision 2
# baseline (speedup 1.0000x reference)
"""Trainium2 Bass kernel for the MichaelsRNN forward pass.

Reference math (per time step t, per batch element b):
    recur = r @ J.T
    inp   = image.T @ I.T + hold.T * S.T
    pre   = 0.9*x + 0.1*(recur + inp + Bb.T)     # Euler step dt/tau = 1/10
    out   = retanh(pre) = tanh(max(pre, 0))
    y     = out[:, :100] @ fc_w.T + fc_b
    carry = (pre, out)

Sharding: data-parallel over the batch axis. B=1024 over 8 cores = 128
batch elements per core, which exactly fills the 128-wide matmul moving
dimension and the SBUF partition count.

On-chip layout (per core, "neuron-partition"):
    state tiles pre/r: [100, 3*128]  partition = within-module neuron id,
                                     free = module*128 + batch
    The whole state update accumulates in one PSUM tile [128, 384] via
    5 matmuls per module:
       3x  J'[m-block, k-block].T (K=100)  @ r[:, k-slice]
       1x  [I'; S'; Bb']-block    (K=22)   @ d_ext (data with ones row)
       1x  0.9*Identity           (K=100)  @ pre[:, m-slice]
    with J' = 0.1*J etc., so PSUM directly holds the new pre.
Then ScalarE does tanh(PSUM) (retanh = relu(tanh) = tanh(relu)), VectorE
does the relu + the pre copy-back + the y bias add.
"""

import numpy as np
import ml_dtypes

import concourse.bass as bass  # noqa: F401  (imported for side effects/types)
import concourse.tile as tile
from concourse import bacc, mybir
from concourse.bass_utils import run_bass_kernel_spmd

# Problem constants (hardcoded per the task contract).
NPM = 100   # neurons per module
NMOD = 3
NN = 300
NF = 20
OUT = 50
T = 500
B = 1024
N_CORES = 8
BS = B // N_CORES      # 128 batch per core
FREE = NMOD * BS       # 384
CH = 20                # time steps per data-in / y-out chunk

W_DT = "bf16"          # matmul dtype for J/I/fc weights + r/data ("fp32"/"bf16")

_BUILD_CACHE: dict = {}


def _w_np():
    return ml_dtypes.bfloat16 if W_DT == "bf16" else np.float32


def _w_mybir():
    return mybir.dt.bfloat16 if W_DT == "bf16" else mybir.dt.float32


def _build_program(n_steps: int):
    """Build + compile the Bass program (value-independent)."""
    wdt = _w_mybir()
    f32 = mybir.dt.float32

    nc = bacc.Bacc(
        "TRN2", target_bir_lowering=False, debug=False, num_devices=N_CORES
    )

    data_ap = nc.dram_tensor("data", [NF + 2, n_steps * BS], wdt, kind="ExternalInput").ap()
    jt_ap = nc.dram_tensor("jt", [NPM, 9 * BS], wdt, kind="ExternalInput").ap()
    ib_ap = nc.dram_tensor("ib", [NF + 2, NMOD * BS], wdt, kind="ExternalInput").ap()
    ident_ap = nc.dram_tensor("ident", [NPM, BS], f32, kind="ExternalInput").ap()
    fct_ap = nc.dram_tensor("fct", [NPM, OUT], wdt, kind="ExternalInput").ap()
    fcb_ap = nc.dram_tensor("fcb", [OUT, 1], f32, kind="ExternalInput").ap()
    pre0_ap = nc.dram_tensor("pre0", [NPM, FREE], f32, kind="ExternalInput").ap()
    r0_ap = nc.dram_tensor("r0", [NPM, FREE], wdt, kind="ExternalInput").ap()
    y_ap = nc.dram_tensor("y", [OUT, n_steps * BS], f32, kind="ExternalOutput").ap()

    with tile.TileContext(nc) as tc:
        with tc.tile_pool(name="const", bufs=1) as const_pool, \
             tc.tile_pool(name="din", bufs=2) as din_pool, \
             tc.tile_pool(name="yout", bufs=2) as yout_pool, \
             tc.tile_pool(name="tmp", bufs=2) as tmp_pool, \
             tc.tile_pool(name="ps_pre", bufs=2, space="PSUM") as ps_pre_pool, \
             tc.tile_pool(name="ps_y", bufs=2, space="PSUM") as ps_y_pool:

            jt = const_pool.tile([NPM, 9 * BS], wdt)
            nc.sync.dma_start(jt[:], jt_ap[:])
            ib = const_pool.tile([NF + 2, NMOD * BS], wdt)
            nc.sync.dma_start(ib[:], ib_ap[:])
            ident = const_pool.tile([NPM, BS], f32)
            nc.sync.dma_start(ident[:], ident_ap[:])
            fct = const_pool.tile([NPM, OUT], wdt)
            nc.sync.dma_start(fct[:], fct_ap[:])
            fcb = const_pool.tile([OUT, 1], f32)
            nc.sync.dma_start(fcb[:], fcb_ap[:])
            pre = const_pool.tile([NPM, FREE], f32)
            nc.sync.dma_start(pre[:], pre0_ap[:])
            r = const_pool.tile([NPM, FREE], wdt)
            nc.sync.dma_start(r[:], r0_ap[:])

            dtile = None
            ybuf = None
            for t in range(n_steps):
                tl = t % CH
                if tl == 0:
                    dtile = din_pool.tile([NF + 2, CH * BS], wdt)
                    nc.sync.dma_start(
                        dtile[:], data_ap[:, t * BS : (t + CH) * BS]
                    )
                    ybuf = yout_pool.tile([OUT, CH * BS], f32)

                ps = ps_pre_pool.tile([128, FREE], f32)
                d_t = dtile[:, tl * BS : (tl + 1) * BS]
                for m in range(NMOD):
                    pslice = ps[:, m * BS : (m + 1) * BS]
                    for k in range(NMOD):
                        nc.tensor.matmul(
                            pslice,
                            jt[:, (k * NMOD + m) * BS : (k * NMOD + m + 1) * BS],
                            r[:, k * BS : (k + 1) * BS],
                            start=(k == 0),
                            stop=False,
                        )
                    nc.tensor.matmul(
                        pslice,
                        ib[:, m * BS : (m + 1) * BS],
                        d_t,
                        start=False,
                        stop=False,
                    )
                    nc.tensor.matmul(
                        pslice,
                        ident[:],
                        pre[:, m * BS : (m + 1) * BS],
                        start=False,
                        stop=True,
                    )

                # y projection from module 0 of the *new* r, computed below —
                # but r for step t's output is retanh(pre_t), so do the
                # elementwise chain first, then the fc matmul.
                th = tmp_pool.tile([NPM, FREE], wdt, tag="th")
                nc.scalar.activation(
                    th[:], ps[0:NPM, :], mybir.ActivationFunctionType.Tanh
                )
                # pre <- PSUM (new pre), WAR on this step's ident matmuls
                nc.vector.tensor_copy(pre[:], ps[0:NPM, :])
                # r <- relu(tanh)
                nc.vector.tensor_scalar_max(r[:], th[:], 0.0)

                psy = ps_y_pool.tile([OUT, BS], f32)
                nc.tensor.matmul(psy[:], fct[:], r[:, 0:BS], start=True, stop=True)
                nc.vector.tensor_scalar_add(
                    ybuf[:, tl * BS : (tl + 1) * BS], psy[:], fcb[:]
                )

                if tl == CH - 1:
                    nc.sync.dma_start(
                        y_ap[:, (t - CH + 1) * BS : (t + 1) * BS], ybuf[:]
                    )

    nc.compile()
    return nc


def _prep_host_inputs(data, J, I, S, Bb, x0, fc_w, fc_b, n_steps: int):
    """Build the per-core input maps (weights replicated, data sharded)."""
    wnp = _w_np()
    f32 = np.float32

    Jp = (0.1 * np.asarray(J, f32))
    Ip = (0.1 * np.asarray(I, f32))
    Sp = (0.1 * np.asarray(S, f32))
    Bbp = (0.1 * np.asarray(Bb, f32))

    jt = np.zeros((NPM, 9, BS), f32)
    for k in range(NMOD):
        for m in range(NMOD):
            blk = Jp[m * NPM : (m + 1) * NPM, k * NPM : (k + 1) * NPM]
            jt[:, k * NMOD + m, :NPM] = blk.T
    jt = jt.reshape(NPM, 9 * BS).astype(wnp)

    ib = np.zeros((NF + 2, NMOD, BS), f32)
    for m in range(NMOD):
        ib[:NF, m, :NPM] = Ip[m * NPM : (m + 1) * NPM, :].T
        ib[NF, m, :NPM] = Sp[m * NPM : (m + 1) * NPM, 0]
        ib[NF + 1, m, :NPM] = Bbp[m * NPM : (m + 1) * NPM, 0]
    ib = ib.reshape(NF + 2, NMOD * BS).astype(wnp)

    ident = np.zeros((NPM, BS), f32)
    ident[np.arange(NPM), np.arange(NPM)] = 0.9

    fct = np.asarray(fc_w, f32).T.astype(wnp)          # [NPM, OUT]
    fcb = np.asarray(fc_b, f32).reshape(OUT, 1)

    x0 = np.asarray(x0, f32)
    pre0 = np.repeat(
        x0.reshape(NMOD, NPM).T[:, :, None], BS, axis=2
    ).reshape(NPM, FREE)
    r0 = np.maximum(np.tanh(pre0), 0.0)

    data = np.asarray(data, f32)[:n_steps]             # [n_steps, NF+1, B]
    dext = np.concatenate(
        [data, np.ones((n_steps, 1, B), f32)], axis=1
    )                                                  # [n_steps, 22, B]
    dext = np.transpose(dext, (1, 0, 2))               # [22, n_steps, B]

    in_maps = []
    for c in range(N_CORES):
        shard = dext[:, :, c * BS : (c + 1) * BS].reshape(NF + 2, n_steps * BS)
        in_maps.append(
            {
                "data": np.ascontiguousarray(shard).astype(wnp),
                "jt": jt,
                "ib": ib,
                "ident": ident,
                "fct": fct,
                "fcb": fcb,
                "pre0": pre0.astype(f32),
                "r0": r0.astype(wnp),
            }
        )
    return in_maps


def _get_program(n_steps: int):
    key = (n_steps, W_DT)
    if key not in _BUILD_CACHE:
        _BUILD_CACHE[key] = _build_program(n_steps)
    return _BUILD_CACHE[key]


def run_sharded(inputs: dict, n_steps: int = T):
    """Compile (cached), run on 8 cores, return per-core raw y + results."""
    nc = _get_program(n_steps)
    in_maps = _prep_host_inputs(n_steps=n_steps, **inputs)
    res = run_bass_kernel_spmd(nc, in_maps, core_ids=list(range(N_CORES)))
    ys = [res.results[c]["y"].reshape(OUT, n_steps, BS) for c in range(N_CORES)]
    y_full = np.stack(ys, axis=0)                      # [8, OUT, n_steps, BS]
    y_full = np.transpose(y_full, (2, 0, 3, 1)).reshape(n_steps, B, OUT)
    return np.ascontiguousarray(y_full, dtype=np.float32)


def kernel(data, J, I, S, Bb, x0, fc_w, fc_b):
    return run_sharded(
        dict(data=data, J=J, I=I, S=S, Bb=Bb, x0=x0, fc_w=fc_w, fc_b=fc_b)
    )
